# revision 3
# baseline (speedup 1.0000x reference)
"""Antialiased bicubic 4x downscale (blur -> bicubic/2, twice) on 8 TRN2 cores.

The whole chain is linear and separable: every stage is M_H (x) M_W acting on
the H/W axes, so the composition collapses to a single 1024->256 banded matrix
T applied on both sides: out = T @ X @ T^T per (batch, channel) image.

Sharding: pure data parallel - batch 16 -> 2 per core, 6 images/core.
Per image on-chip:
  pass 1: Yt[w, i] = sum_p X[p, w] * Tt[p, i]   (X stationary, Tt moving)
  pass 2: Z[i, j]  = sum_q Yt[q, i] * Tt[q, j]  (Yt stationary, Tt moving)
Both passes accumulate over 8 K-chunks of 128 in PSUM; float32r matmuls run
at 1 cycle/row since the moving free dim is 256.
"""

import numpy as np

import concourse.bacc as bacc
import concourse.mybir as mybir
import concourse.tile as tile
from concourse.bass_utils import run_bass_kernel_spmd

SIGMA = 0.66
BICUBIC_W = np.array([-0.09375, 0.59375, 0.59375, -0.09375], dtype=np.float64)

N_CORES = 8
B, C, H, W = 16, 3, 1024, 1024
HO = H // 4
IMGS = (B // N_CORES) * C  # 6 images per core

F32 = mybir.dt.float32
F32R = mybir.dt.float32r


def _gauss_matrix(n):
    x = np.arange(3, dtype=np.float32) - np.float32(1.0)
    k = np.exp(np.float32(-0.5) * (x / np.float32(SIGMA)) ** 2)
    k = (k / k.sum()).astype(np.float64)
    G = np.zeros((n, n))
    for t in range(3):
        G += k[t] * np.eye(n, n, t - 1)
    return G


def _down_matrix(n):
    # out[i] = sum_t w[t] * x[clamp(2i + t - 1, 0, n-1)]
    m = n // 2
    D = np.zeros((m, n))
    for i in range(m):
        for t in range(4):
            j = min(max(2 * i + t - 1, 0), n - 1)
            D[i, j] += BICUBIC_W[t]
    return D


def build_T():
    T = _down_matrix(H // 2) @ _gauss_matrix(H // 2) @ _down_matrix(H) @ _gauss_matrix(H)
    return T.astype(np.float32)  # [256, 1024]


def _build_graph():
    nc = bacc.Bacc("TRN2", target_bir_lowering=False, debug=False)
    x = nc.dram_tensor("x", [IMGS, H, W], F32R, kind="ExternalInput").ap()
    t = nc.dram_tensor("t", [H, HO], F32R, kind="ExternalInput").ap()
    out = nc.dram_tensor("out", [IMGS, HO, HO], F32, kind="ExternalOutput").ap()

    with tile.TileContext(nc) as tc:
        with (
            tc.tile_pool(name="const", bufs=1) as cpool,
            tc.tile_pool(name="xin", bufs=2) as xpool,
            tc.tile_pool(name="yt", bufs=2) as ypool,
            tc.tile_pool(name="zout", bufs=2) as zpool,
            tc.tile_pool(name="ps1", bufs=4, space="PSUM") as ps1,
            tc.tile_pool(name="ps2", bufs=2, space="PSUM") as ps2,
        ):
            tt = cpool.tile([128, 8, HO], F32R)
            nc.sync.dma_start(out=tt[:], in_=t.rearrange("(c p) n -> p c n", p=128))
            for img in range(IMGS):
                xt = xpool.tile([128, 8, W], F32R)
                nc.sync.dma_start(
                    out=xt[:], in_=x[img].rearrange("(c p) w -> p c w", p=128)
                )
                yt = ypool.tile([128, 8, HO], F32R)
                for wc in range(8):
                    acc = ps1.tile([128, HO], F32)
                    for pc in range(8):
                        nc.tensor.matmul(
                            acc[:],
                            xt[:, pc, wc * 128 : (wc + 1) * 128],
                            tt[:, pc, :],
                            start=(pc == 0),
                            stop=(pc == 7),
                        )
                    if wc % 2 == 0:
                        nc.vector.tensor_copy(yt[:, wc, :], acc[:])
                    else:
                        nc.scalar.copy(yt[:, wc, :], acc[:])
                z = zpool.tile([128, 2, HO], F32)
                for ih in range(2):
                    acc2 = ps2.tile([128, HO], F32)
                    for qc in range(8):
                        nc.tensor.matmul(
                            acc2[:],
                            yt[:, qc, ih * 128 : (ih + 1) * 128],
                            tt[:, qc, :],
                            start=(qc == 0),
                            stop=(qc == 7),
                        )
                    nc.vector.tensor_copy(z[:, ih, :], acc2[:])
                nc.sync.dma_start(
                    out=out[img].rearrange("(c p) j -> p c j", p=128), in_=z[:]
                )
    nc.compile()
    return nc


_GRAPH = None


def _get_graph():
    global _GRAPH
    if _GRAPH is None:
        _GRAPH = _build_graph()
    return _GRAPH


def run(x, **spmd_kwargs):
    x = np.ascontiguousarray(np.asarray(x, dtype=np.float32))
    assert x.shape == (B, C, H, W)
    nc = _get_graph()
    tt_host = np.ascontiguousarray(build_T().T)  # [1024, 256]
    per_core = B // N_CORES
    in_maps = [
        {
            "x": x[i * per_core : (i + 1) * per_core].reshape(IMGS, H, W),
            "t": tt_host,
        }
        for i in range(N_CORES)
    ]
    res = run_bass_kernel_spmd(nc, in_maps, core_ids=list(range(N_CORES)), **spmd_kwargs)
    out = np.concatenate(
        [r["out"].reshape(per_core, C, HO, HO) for r in res.results], axis=0
    )
    return out, res


def kernel(x):
    out, _ = run(x)
    return out
